# revision 25
# baseline (speedup 1.0000x reference)
"""Bidirectional LSTM (TF BasicLSTMCell semantics) on 8 Trainium2 NeuronCores.

Sharding: data-parallel on (batch, direction). Cores 0-3 run the forward
LSTM over 16 batches each; cores 4-7 run the backward LSTM over the
host-reversed sequences for 16 batches each. Weights are replicated per
direction. All 8 cores run one identical Bass program; only input data
differs per core.

Per-core layout is "gate-major": features/gates live on SBUF partitions,
the 16 sequences live on the free axis. The recurrent matmul keeps W_h
stationary (bf16) and streams h (N=16 moving); at this shape the PE is
instruction-issue bound (~34ns per LDWEIGHTS+MATMUL pair, 144 pairs per
step), not weight-load-bandwidth bound (verified: fp8 weights give no
speedup). The recurrence runs as two PSUM passes over k-halves of h and
the elementwise chain is split per half, so the next step's first pass
starts as soon as half 0 of the new h is cast. The input projection
x @ W_x is precomputed in 16-step stripes on the otherwise-idle PE
moving port. Outputs are transposed back to batch-major with PE
transposes at stripe boundaries; the final state is extracted host-side
from outputs[s, seq_len-1] (dynamic_rnn freezes state at seq_len).

The time loop is a Tile For_i over stripe PAIRS (so xp/xts buffers
ping-pong statically inside the body); the loop back-edge resets
semaphores, which a fully-unrolled 512-step program would overflow.
DRAM tensors are padded by one/two stripes so the first/last iteration
needs no control flow: iteration 0 "flushes" garbage into a leading pad
stripe of the output, and the last iteration prefetches zero-padded
input stripes.
"""

import sys

sys.path.insert(0, "/opt/trn_rl_repo")

import numpy as np
import ml_dtypes

B, T, D, H = 64, 512, 512, 768
FORGET_BIAS = 1.0

S = 16         # sequences per core
ST = 16        # steps per stripe
KT_X = D // 128    # 4  k-tiles of the input projection
KT_H = H // 128    # 6  k-tiles of the recurrence
MT = 4 * H // 128  # 24 gate m-tiles

_PROGRAM_CACHE = {}
W_DTYPE = "bfloat16"  # perf-probe knob; "float8e4" halves LDWEIGHTS time
NO_CHAIN = False      # perf-probe knob: skip elementwise chain (wrong results)
EXTRA_NOP = 0         # perf-probe knob: perturb program to bust the NEFF cache


def _build_program(t_steps):
    import concourse.bass as bass
    import concourse.mybir as mybir
    import concourse.tile as tile
    from concourse import bacc
    from concourse.bass import ds, ts
    from concourse.masks import make_identity

    f32 = mybir.dt.float32
    bf16 = mybir.dt.bfloat16
    wdt = getattr(mybir.dt, W_DTYPE)
    u8 = mybir.dt.uint8
    AF = mybir.ActivationFunctionType
    ET = mybir.EngineType

    n_stripes = t_steps // ST
    assert n_stripes * ST == t_steps and n_stripes % 2 == 0

    nc = bacc.Bacc(
        "TRN2", target_bir_lowering=False, debug=False, num_devices=8
    )

    # xts is padded with 2 zero stripes at the end (loop prefetch overrun);
    # outd has one garbage pad stripe at the START (iteration-0 flush).
    wt_d = nc.dram_tensor("wt", [128, KT_X + KT_H, 4 * H], wdt, kind="ExternalInput")
    b_d = nc.dram_tensor("bcol", [128, MT], f32, kind="ExternalInput")
    sl_d = nc.dram_tensor("slen", [128, KT_H, S], f32, kind="ExternalInput")
    xt_d = nc.dram_tensor(
        "xts", [128, KT_X, (t_steps + 2 * ST) * S], bf16, kind="ExternalInput"
    )
    out_d = nc.dram_tensor("outd", [S, t_steps + ST, H], f32, kind="ExternalOutput")

    wt_a, b_a, sl_a, xt_a = wt_d.ap(), b_d.ap(), sl_d.ap(), xt_d.ap()
    out_a = out_d.ap()

    with tile.TileContext(nc) as tc:
        with (
            tc.tile_pool(name="const", bufs=1) as constp,
            tc.tile_pool(name="xtsp", bufs=1) as xtsp,
            tc.tile_pool(name="xpp", bufs=1) as xpp,
            tc.tile_pool(name="outsp", bufs=1) as outsp,
            tc.tile_pool(name="osbp", bufs=2) as osbp,
            tc.tile_pool(name="workp", bufs=2) as workp,
            tc.tile_pool(name="psg", bufs=2, space=bass.MemorySpace.PSUM) as psgp,
            tc.tile_pool(name="psx", bufs=2, space=bass.MemorySpace.PSUM) as psxp,
            tc.tile_pool(name="pst", bufs=2, space=bass.MemorySpace.PSUM) as pstp,
        ):
            wt = constp.tile([128, KT_X + KT_H, 4 * H], wdt, tag="wt", name="wt_sb")
            bcol = constp.tile([128, MT], f32, tag="bcol", name="bcol_sb")
            slen = constp.tile([128, KT_H, S], f32, tag="slen", name="slen_sb")
            ident = constp.tile([128, 128], f32, tag="ident", name="ident_sb")
            c_sb = constp.tile([128, KT_H, S], f32, tag="c", name="c_sb")
            h_bf = constp.tile([128, KT_H, S], bf16, tag="hbf", name="hbf_sb")
            t_vec = constp.tile([128, 1], f32, tag="tvec", name="tvec_sb")

            # static double buffers (ping-pong inside the For_i body)
            xtsA = xtsp.tile([128, KT_X, ST * S], bf16, tag="xtsA", name="xtsA_sb")
            xtsB = xtsp.tile([128, KT_X, ST * S], bf16, tag="xtsB", name="xtsB_sb")
            # gates layout: [128, ktile(6), gate(4)*S] so k-half slices stay
            # 2-D in the free dims (TensorTensor APs allow at most 2)
            xpA = xpp.tile([128, KT_H, ST, 4 * S], f32, tag="xpA", name="xpA_sb")
            xpB = xpp.tile([128, KT_H, ST, 4 * S], f32, tag="xpB", name="xpB_sb")
            outA = outsp.tile([128, KT_H, ST, S], f32, tag="outA", name="outA_sb")
            outB = outsp.tile([128, KT_H, ST, S], f32, tag="outB", name="outB_sb")

            nc.sync.dma_start(wt[:], wt_a[:])
            nc.sync.dma_start(bcol[:], b_a[:])
            nc.sync.dma_start(slen[:], sl_a[:])
            make_identity(nc, ident[:])
            nc.gpsimd.memset(c_sb[:], 0.0)
            nc.gpsimd.memset(h_bf[:], 0.0)
            nc.gpsimd.memset(t_vec[:], 0.0)
            for _ in range(EXTRA_NOP):
                nc.gpsimd.memset(t_vec[:], 0.0)

            def load_xts(dst, stripe_off):
                # stripe_off: element offset expression (stripe_idx * ST * S)
                nc.sync.dma_start(dst[:], xt_a[:, :, ds(stripe_off, ST * S)])

            def xp_mgroup(m, xp_tile, xts_tile):
                # xp[:, g, kt, :] = W_x[:, m-tile].T @ x_stripe + b[m-tile]
                psx_t = psxp.tile([128, ST * S], f32, tag="psx", name="psx_t")
                for k in range(KT_X):
                    nc.tensor.matmul(
                        psx_t[:],
                        wt[:, k, ts(m, 128)],
                        xts_tile[:, k, :],
                        start=(k == 0),
                        stop=(k == KT_X - 1),
                    )
                g, kt = divmod(m, KT_H)
                nc.scalar.activation(
                    xp_tile[:, kt, :, ts(g, S)],
                    psx_t[:].rearrange("p (t s) -> p t s", s=S),
                    AF.Identity,
                    bias=bcol[:, ds(m, 1)],
                )

            def flush_unit(out_tile, osb_t, th, kt):
                # transpose [128h, (8t,16s)] -> [(8t,16s), 128h]
                pst_t = pstp.tile([128, 128], f32, tag="pst", name="pst_t")
                nc.tensor.transpose(
                    pst_t[:], out_tile[:, kt, ds(th * 8, 8), :], ident[:]
                )
                nc.vector.tensor_copy(osb_t[:, ts(kt, 128)], pst_t[:])

            def flush_dma(osb_t, row_off):
                nc.sync.dma_start(
                    out_a[0:S, ds(row_off, 8), :].rearrange("s t h -> t s h"),
                    osb_t[:],
                )

            def stripe_body(xp_cur, xp_nxt, xts_nxt, out_tile, flush_src, flush_row):
                """One stripe of 16 recurrence steps.

                xp_cur: xp for this stripe. xp_nxt/xts_nxt: compute next
                stripe's xp in the PE bubbles. flush_src/flush_row: previous
                stripe's output tile to transpose+store (row_off expression).
                """
                nc.gpsimd.memset(out_tile[:], 0.0)
                osb = {}
                for tl in range(ST):
                    # mask_t = (t < seq_len); t_vec counts steps on gpsimd
                    mask = workp.tile([128, KT_H, S], u8, tag="mask", name="mask_t")
                    nc.vector.tensor_tensor(
                        mask[:],
                        slen[:],
                        t_vec[:].to_broadcast((128, KT_H, S)),
                        mybir.AluOpType.is_gt,
                    )
                    nc.vector.tensor_scalar_add(t_vec[:], t_vec[:], 1.0)

                    # recurrent matmuls, two contraction passes so the next
                    # step's pass A only waits on half 0 of the new h.
                    # Separate PSUM tiles: a start=True clears has_written
                    # bits bank-wide, so passes must not share a bank.
                    psgA_t = psgp.tile(
                        [128, KT_H, 4 * S], f32, tag="psgA", name="psgA_t"
                    )
                    psgB_t = psgp.tile(
                        [128, KT_H, 4 * S], f32, tag="psgB", name="psgB_t"
                    )
                    for m in range(MT):
                        g, kt = divmod(m, KT_H)
                        for k in range(3):
                            nc.tensor.matmul(
                                psgA_t[:, kt, ts(g, S)],
                                wt[:, KT_X + k, ts(m, 128)],
                                h_bf[:, k, :],
                                start=(k == 0),
                                stop=(k == 2),
                            )
                    for m in range(MT):
                        g, kt = divmod(m, KT_H)
                        for k in range(3, KT_H):
                            nc.tensor.matmul(
                                psgB_t[:, kt, ts(g, S)],
                                wt[:, KT_X + k, ts(m, 128)],
                                h_bf[:, k, :],
                                start=(k == 3),
                                stop=(k == KT_H - 1),
                            )

                    # fill the PE bubble: next stripe's xp m-groups
                    if xp_nxt is not None:
                        for m in range(24 * tl // ST, 24 * (tl + 1) // ST):
                            xp_mgroup(m, xp_nxt, xts_nxt)

                    # fill the PE bubble: transpose-flush of the previous stripe
                    if flush_src is not None and 2 <= tl <= 14:
                        u = tl - 2
                        if u < 12:
                            th, kt = u // 6, u % 6
                            if kt == 0:
                                osb[th] = osbp.tile(
                                    [128, H], f32, tag="osb", name="osb_t"
                                )
                            flush_unit(flush_src, osb[th], th, kt)
                            if kt == 5:
                                flush_dma(osb[th], flush_row + th * 8)

                    if NO_CHAIN:
                        continue
                    # elementwise chain, split into two k-halves so h_bf's
                    # first half is ready as early as possible
                    gt = workp.tile([128, KT_H, 4 * S], f32, tag="gt", name="gt_t")
                    ac = workp.tile([128, KT_H, 4 * S], f32, tag="ac", name="ac_t")
                    t1 = workp.tile([128, KT_H, S], f32, tag="t1", name="t1_t")
                    t2 = workp.tile([128, KT_H, S], f32, tag="t2", name="t2_t")
                    tct = workp.tile([128, KT_H, S], f32, tag="tct", name="tct_t")
                    ht = workp.tile([128, KT_H, S], f32, tag="ht", name="ht_t")
                    for hx in range(2):
                        kr = slice(3 * hx, 3 * hx + 3)
                        # gates = psumA + psumB + xp (bias folded into xp);
                        # each add reads at most one PSUM operand
                        nc.vector.tensor_add(
                            gt[:, kr, :], psgA_t[:, kr, :], xp_cur[:, kr, tl, :]
                        )
                        nc.vector.tensor_add(
                            gt[:, kr, :], psgB_t[:, kr, :], gt[:, kr, :]
                        )
                        nc.scalar.activation(
                            ac[:, kr, ts(0, S)], gt[:, kr, ts(0, S)], AF.Sigmoid
                        )
                        nc.scalar.activation(
                            ac[:, kr, ts(1, S)], gt[:, kr, ts(1, S)], AF.Tanh
                        )
                        nc.scalar.activation(
                            ac[:, kr, ts(2, S)], gt[:, kr, ts(2, S)], AF.Sigmoid
                        )
                        nc.scalar.activation(
                            ac[:, kr, ts(3, S)], gt[:, kr, ts(3, S)], AF.Sigmoid
                        )
                        # c = mask ? (c * f + i * j) : c
                        nc.vector.tensor_mul(
                            t1[:, kr, :], ac[:, kr, ts(0, S)], ac[:, kr, ts(1, S)]
                        )
                        nc.vector.tensor_mul(
                            t2[:, kr, :], c_sb[:, kr, :], ac[:, kr, ts(2, S)]
                        )
                        nc.vector.tensor_add(t2[:, kr, :], t2[:, kr, :], t1[:, kr, :])
                        nc.vector.copy_predicated(
                            c_sb[:, kr, :], mask[:, kr, :], t2[:, kr, :]
                        )
                        # h = mask ? tanh(c) * o : h (h_bf updated directly)
                        nc.scalar.activation(tct[:, kr, :], c_sb[:, kr, :], AF.Tanh)
                        nc.vector.tensor_mul(
                            ht[:, kr, :], tct[:, kr, :], ac[:, kr, ts(3, S)]
                        )
                        nc.vector.copy_predicated(
                            h_bf[:, kr, :], mask[:, kr, :], ht[:, kr, :]
                        )
                        nc.vector.copy_predicated(
                            out_tile[:, kr, tl, :], mask[:, kr, :], ht[:, kr, :]
                        )

            # Prologue: stripes 0,1 staged; xp for stripe 0
            load_xts(xtsA, 0)
            load_xts(xtsB, ST * S)
            for m in range(MT):
                xp_mgroup(m, xpA, xtsA)

            with tc.For_i(
                0,
                n_stripes,
                2,
                hint_engines=(ET.PE, ET.DVE, ET.Activation, ET.SP, ET.Pool),
            ) as iv:
                # iv = even stripe index a; stripes (a, a+1) this iteration.
                # xtsA holds x(a) (consumed last iter) -> refill with x(a+2)
                # now; xtsB (holding x(a+1)) is still needed by stripe a's
                # xp-build, so its refill with x(a+3) is emitted after.
                load_xts(xtsA, iv * (ST * S) + 2 * ST * S)
                # stripe a: consume xpA; build xp(a+1) in xpB from xtsB;
                # flush previous iteration's outB (stripe a-1) to rows
                # (a-1+1)*ST = iv*ST (iteration 0 writes the pad stripe).
                stripe_body(xpA, xpB, xtsB, outA, outB, iv * ST)
                load_xts(xtsB, iv * (ST * S) + 3 * ST * S)
                # stripe a+1: consume xpB; build xp(a+2) in xpA from the
                # refilled xtsA; flush outA (stripe a) to rows (a+1)*ST.
                stripe_body(xpB, xpA, xtsA, outB, outA, iv * ST + ST)

            # Epilogue: flush the last stripe (n_stripes-1, in outB) + state
            for th in range(2):
                osb_t = osbp.tile([128, H], f32, tag="osb", name="osb_t")
                for kt in range(KT_H):
                    flush_unit(outB, osb_t, th, kt)
                flush_dma(osb_t, n_stripes * ST + th * 8)


    nc.compile()
    return nc


def _get_program(t_steps):
    if t_steps not in _PROGRAM_CACHE:
        _PROGRAM_CACHE[t_steps] = _build_program(t_steps)
    return _PROGRAM_CACHE[t_steps]


def _reverse_sequence_np(x, seq_len):
    # mirrors tf.reverse_sequence along axis 1
    t = np.arange(x.shape[1])
    idx = np.where(
        t[None, :] < seq_len[:, None], seq_len[:, None] - 1 - t[None, :], t[None, :]
    )
    return np.take_along_axis(x, idx[:, :, None], axis=1)


def _core_inputs(x_blk, w_np, b_np, sl_blk, t_steps):
    """Build one core's input map from a [16, T, D] fp32 block."""
    bf = ml_dtypes.bfloat16
    # xts[p, k, t*16+s] = x[s, t, 128k+p]; pad 2 stripes of zeros in t
    xts = np.zeros((128, KT_X, (t_steps + 2 * ST) * S), dtype=bf)
    xts[:, :, : t_steps * S] = (
        x_blk.transpose(2, 1, 0).reshape(KT_X, 128, t_steps * S).transpose(1, 0, 2)
    ).astype(bf)
    # wt[p, k, g] = W[128k+p, g]
    import concourse.mybir as _mybir

    wnp_dt = _mybir.dt.np(getattr(_mybir.dt, W_DTYPE))
    wt = np.ascontiguousarray(
        w_np.reshape(KT_X + KT_H, 128, 4 * H).transpose(1, 0, 2)
    ).astype(wnp_dt)
    b_eff = b_np.astype(np.float64).copy()
    b_eff[2 * H : 3 * H] += FORGET_BIAS
    bcol = np.ascontiguousarray(b_eff.reshape(MT, 128).T).astype(np.float32)
    slen = np.broadcast_to(
        sl_blk.astype(np.float32)[None, None, :], (128, KT_H, S)
    ).copy()
    return {"wt": wt, "bcol": bcol, "slen": slen, "xts": xts}


_RUNNER_CACHE = {}


def _get_runner(t_steps):
    """Build (once) a jitted 8-core shard_map dispatcher for the program.

    Modeled on bass2jax.run_bass_via_pjrt, but the jitted callable and the
    donated-zero output operands are cached so repeat kernel() calls pay
    only input transfer + device execution, not XLA/NEFF recompilation.
    The kernel writes every output element, so donation is not needed.
    """
    if t_steps in _RUNNER_CACHE:
        return _RUNNER_CACHE[t_steps]

    import jax
    from jax.sharding import Mesh, PartitionSpec, NamedSharding
    from jax.experimental.shard_map import shard_map
    import concourse.mybir as mybir
    from concourse import bass2jax
    from concourse.bass2jax import _bass_exec_p, install_neuronx_cc_hook

    nc = _get_program(t_steps)
    install_neuronx_cc_hook()
    n_cores = 8

    partition_name = nc.partition_id_tensor.name if nc.partition_id_tensor else None
    in_names, out_names, out_avals, zero_outs = [], [], [], []
    for alloc in nc.m.functions[0].allocations:
        if not isinstance(alloc, mybir.MemoryLocationSet):
            continue
        name = alloc.memorylocations[0].name
        if alloc.kind == "ExternalInput":
            if name != partition_name:
                in_names.append(name)
        elif alloc.kind == "ExternalOutput":
            out_names.append(name)
            shape = tuple(alloc.tensor_shape)
            dtype = mybir.dt.np(alloc.dtype)
            out_avals.append(jax.core.ShapedArray(shape, dtype))
            zero_outs.append(np.zeros(shape, dtype))
    n_params = len(in_names)
    all_in_names = in_names + out_names

    def _body(*args):
        operands = list(args)
        if partition_name is not None:
            operands.append(bass2jax.partition_id_tensor())
            names = all_in_names + [partition_name]
        else:
            names = all_in_names
        outs = _bass_exec_p.bind(
            *operands,
            out_avals=tuple(out_avals),
            in_names=tuple(names),
            out_names=tuple(out_names),
            lowering_input_output_aliases=(),
            sim_require_finite=True,
            sim_require_nnan=True,
            nc=nc,
        )
        return tuple(outs)

    devices = jax.devices()[:n_cores]
    mesh = Mesh(np.asarray(devices), ("core",))
    sharded = jax.jit(
        shard_map(
            _body,
            mesh=mesh,
            in_specs=(PartitionSpec("core"),) * (n_params + len(out_names)),
            out_specs=(PartitionSpec("core"),) * len(out_names),
            check_rep=False,
        )
    )
    sh = NamedSharding(mesh, PartitionSpec("core"))
    zero_args = [
        jax.device_put(
            np.zeros((n_cores * z.shape[0], *z.shape[1:]), z.dtype), sh
        )
        for z in zero_outs
    ]

    def run(in_maps, raw=False):
        concat_in = [
            np.concatenate([in_maps[c][name] for c in range(n_cores)], axis=0)
            for name in in_names
        ]
        dev_in = [jax.device_put(a, sh) for a in concat_in]
        if raw:
            # benchmarking hook: async-dispatch against resident inputs
            return lambda: sharded(*dev_in, *zero_args)
        out_arrs = sharded(*dev_in, *zero_args)
        return [
            {
                name: np.asarray(out_arrs[i]).reshape(
                    n_cores, *out_avals[i].shape
                )[c]
                for i, name in enumerate(out_names)
            }
            for c in range(n_cores)
        ]

    _RUNNER_CACHE[t_steps] = run
    return run


def kernel(seq, seq_len, W_fw, b_fw, W_bw, b_bw):
    seq = np.asarray(seq, dtype=np.float32)
    seq_len = np.asarray(seq_len, dtype=np.int32)
    W_fw = np.asarray(W_fw, dtype=np.float32)
    b_fw = np.asarray(b_fw, dtype=np.float32)
    W_bw = np.asarray(W_bw, dtype=np.float32)
    b_bw = np.asarray(b_bw, dtype=np.float32)

    b_sz, t_steps, _ = seq.shape
    assert b_sz == B and seq.shape[2] == D

    run = _get_runner(t_steps)

    seq_rev = _reverse_sequence_np(seq, seq_len)

    in_maps = []
    for c in range(8):
        blk = slice(16 * (c % 4), 16 * (c % 4) + S)
        if c < 4:
            in_maps.append(_core_inputs(seq[blk], W_fw, b_fw, seq_len[blk], t_steps))
        else:
            in_maps.append(
                _core_inputs(seq_rev[blk], W_bw, b_bw, seq_len[blk], t_steps)
            )

    results = run(in_maps)

    outputs = np.zeros((B, t_steps, 2 * H), dtype=np.float32)
    state_h = np.zeros((B, 2 * H), dtype=np.float32)
    idx = np.arange(S)
    for c in range(8):
        blk = slice(16 * (c % 4), 16 * (c % 4) + S)
        o = results[c]["outd"][:, ST:, :]
        # final state == last valid output of the raw recurrence
        st = o[idx, seq_len[blk] - 1, :]
        if c < 4:
            outputs[blk, :, :H] = o
            state_h[blk, :H] = st
        else:
            outputs[blk, :, H:] = _reverse_sequence_np(o, seq_len[blk])
            state_h[blk, H:] = st
    return outputs, state_h


# revision 26
# speedup vs baseline: 1.0850x; 1.0850x over previous
"""Bidirectional LSTM (TF BasicLSTMCell semantics) on 8 Trainium2 NeuronCores.

Sharding: data-parallel on (batch, direction). Cores 0-3 run the forward
LSTM over 16 batches each; cores 4-7 run the backward LSTM over the
host-reversed sequences for 16 batches each. Weights are replicated per
direction. All 8 cores run one identical Bass program; only input data
differs per core.

Per-core layout is "gate-major": features/gates live on SBUF partitions,
the 16 sequences live on the free axis. The recurrent matmul keeps W_h
stationary (bf16) and streams h (N=16 moving); at this shape the PE is
instruction-issue bound (~34ns per LDWEIGHTS+MATMUL pair, 144 pairs per
step), not weight-load-bandwidth bound (verified: fp8 weights give no
speedup). The recurrence runs as two PSUM passes over k-halves of h and
the elementwise chain is split per half, so the next step's first pass
starts as soon as half 0 of the new h is cast. The input projection
x @ W_x is precomputed in 16-step stripes on the otherwise-idle PE
moving port. Outputs are transposed back to batch-major with PE
transposes at stripe boundaries; the final state is extracted host-side
from outputs[s, seq_len-1] (dynamic_rnn freezes state at seq_len).

The time loop is a Tile For_i over stripe PAIRS (so xp/xts buffers
ping-pong statically inside the body); the loop back-edge resets
semaphores, which a fully-unrolled 512-step program would overflow.
DRAM tensors are padded by one/two stripes so the first/last iteration
needs no control flow: iteration 0 "flushes" garbage into a leading pad
stripe of the output, and the last iteration prefetches zero-padded
input stripes.
"""

import sys

sys.path.insert(0, "/opt/trn_rl_repo")

import numpy as np
import ml_dtypes

B, T, D, H = 64, 512, 512, 768
FORGET_BIAS = 1.0

S = 16         # sequences per core
ST = 16        # steps per stripe
KT_X = D // 128    # 4  k-tiles of the input projection
KT_H = H // 128    # 6  k-tiles of the recurrence
MT = 4 * H // 128  # 24 gate m-tiles

_PROGRAM_CACHE = {}
W_DTYPE = "bfloat16"  # perf-probe knob; "float8e4" halves LDWEIGHTS time
NO_CHAIN = False      # perf-probe knob: skip elementwise chain (wrong results)
EXTRA_NOP = 0         # perf-probe knob: perturb program to bust the NEFF cache


def _build_program(t_steps):
    import concourse.bass as bass
    import concourse.mybir as mybir
    import concourse.tile as tile
    from concourse import bacc
    from concourse.bass import ds, ts
    from concourse.masks import make_identity

    f32 = mybir.dt.float32
    bf16 = mybir.dt.bfloat16
    wdt = getattr(mybir.dt, W_DTYPE)
    u8 = mybir.dt.uint8
    AF = mybir.ActivationFunctionType
    ET = mybir.EngineType

    n_stripes = t_steps // ST
    assert n_stripes * ST == t_steps and n_stripes % 2 == 0

    nc = bacc.Bacc(
        "TRN2", target_bir_lowering=False, debug=False, num_devices=8
    )

    # xts is padded with 2 zero stripes at the end (loop prefetch overrun);
    # outd has one garbage pad stripe at the START (iteration-0 flush).
    wt_d = nc.dram_tensor("wt", [128, KT_X + KT_H, 4 * H], wdt, kind="ExternalInput")
    b_d = nc.dram_tensor("bcol", [128, MT], f32, kind="ExternalInput")
    sl_d = nc.dram_tensor("slen", [128, KT_H, S], f32, kind="ExternalInput")
    xt_d = nc.dram_tensor(
        "xts", [128, KT_X, (t_steps + 2 * ST) * S], bf16, kind="ExternalInput"
    )
    out_d = nc.dram_tensor("outd", [S, t_steps + ST, H], f32, kind="ExternalOutput")

    wt_a, b_a, sl_a, xt_a = wt_d.ap(), b_d.ap(), sl_d.ap(), xt_d.ap()
    out_a = out_d.ap()

    with tile.TileContext(nc) as tc:
        with (
            tc.tile_pool(name="const", bufs=1) as constp,
            tc.tile_pool(name="xtsp", bufs=1) as xtsp,
            tc.tile_pool(name="xpp", bufs=1) as xpp,
            tc.tile_pool(name="outsp", bufs=1) as outsp,
            tc.tile_pool(name="osbp", bufs=2) as osbp,
            tc.tile_pool(name="workp", bufs=2) as workp,
            tc.tile_pool(name="psg", bufs=2, space=bass.MemorySpace.PSUM) as psgp,
            tc.tile_pool(name="psx", bufs=2, space=bass.MemorySpace.PSUM) as psxp,
            tc.tile_pool(name="pst", bufs=2, space=bass.MemorySpace.PSUM) as pstp,
        ):
            wt = constp.tile([128, KT_X + KT_H, 4 * H], wdt, tag="wt", name="wt_sb")
            bcol = constp.tile([128, MT], f32, tag="bcol", name="bcol_sb")
            slen = constp.tile([128, KT_H, S], f32, tag="slen", name="slen_sb")
            ident = constp.tile([128, 128], f32, tag="ident", name="ident_sb")
            c_sb = constp.tile([128, KT_H, S], f32, tag="c", name="c_sb")
            h_bf = constp.tile([128, KT_H, S], bf16, tag="hbf", name="hbf_sb")
            t_vec = constp.tile([128, 1], f32, tag="tvec", name="tvec_sb")

            # static double buffers (ping-pong inside the For_i body)
            xtsA = xtsp.tile([128, KT_X, ST * S], bf16, tag="xtsA", name="xtsA_sb")
            xtsB = xtsp.tile([128, KT_X, ST * S], bf16, tag="xtsB", name="xtsB_sb")
            # gates layout: [128, ktile(6), gate(4)*S] so k-half slices stay
            # 2-D in the free dims (TensorTensor APs allow at most 2)
            xpA = xpp.tile([128, KT_H, ST, 4 * S], f32, tag="xpA", name="xpA_sb")
            xpB = xpp.tile([128, KT_H, ST, 4 * S], f32, tag="xpB", name="xpB_sb")
            outA = outsp.tile([128, KT_H, ST, S], f32, tag="outA", name="outA_sb")
            outB = outsp.tile([128, KT_H, ST, S], f32, tag="outB", name="outB_sb")

            nc.sync.dma_start(wt[:], wt_a[:])
            nc.sync.dma_start(bcol[:], b_a[:])
            nc.sync.dma_start(slen[:], sl_a[:])
            make_identity(nc, ident[:])
            nc.gpsimd.memset(c_sb[:], 0.0)
            nc.gpsimd.memset(h_bf[:], 0.0)
            nc.gpsimd.memset(t_vec[:], 0.0)
            for _ in range(EXTRA_NOP):
                nc.gpsimd.memset(t_vec[:], 0.0)

            def load_xts(dst, stripe_off):
                # stripe_off: element offset expression (stripe_idx * ST * S)
                nc.sync.dma_start(dst[:], xt_a[:, :, ds(stripe_off, ST * S)])

            def xp_mgroup(m, xp_tile, xts_tile):
                # xp[:, g, kt, :] = W_x[:, m-tile].T @ x_stripe + b[m-tile]
                psx_t = psxp.tile([128, ST * S], f32, tag="psx", name="psx_t")
                for k in range(KT_X):
                    nc.tensor.matmul(
                        psx_t[:],
                        wt[:, k, ts(m, 128)],
                        xts_tile[:, k, :],
                        start=(k == 0),
                        stop=(k == KT_X - 1),
                    )
                g, kt = divmod(m, KT_H)
                nc.scalar.activation(
                    xp_tile[:, kt, :, ts(g, S)],
                    psx_t[:].rearrange("p (t s) -> p t s", s=S),
                    AF.Identity,
                    bias=bcol[:, ds(m, 1)],
                )

            def flush_unit(out_tile, osb_t, th, kt):
                # transpose [128h, (8t,16s)] -> [(8t,16s), 128h]
                pst_t = pstp.tile([128, 128], f32, tag="pst", name="pst_t")
                nc.tensor.transpose(
                    pst_t[:], out_tile[:, kt, ds(th * 8, 8), :], ident[:]
                )
                nc.vector.tensor_copy(osb_t[:, ts(kt, 128)], pst_t[:])

            def flush_dma(osb_t, row_off):
                nc.sync.dma_start(
                    out_a[0:S, ds(row_off, 8), :].rearrange("s t h -> t s h"),
                    osb_t[:],
                )

            def stripe_body(xp_cur, xp_nxt, xts_nxt, out_tile, flush_src, flush_row):
                """One stripe of 16 recurrence steps.

                xp_cur: xp for this stripe. xp_nxt/xts_nxt: compute next
                stripe's xp in the PE bubbles. flush_src/flush_row: previous
                stripe's output tile to transpose+store (row_off expression).
                """
                nc.gpsimd.memset(out_tile[:], 0.0)
                osb = {}
                for tl in range(ST):
                    # mask_t = (t < seq_len); t_vec counts steps on gpsimd
                    mask = workp.tile([128, KT_H, S], u8, tag="mask", name="mask_t")
                    nc.vector.tensor_tensor(
                        mask[:],
                        slen[:],
                        t_vec[:].to_broadcast((128, KT_H, S)),
                        mybir.AluOpType.is_gt,
                    )
                    nc.vector.tensor_scalar_add(t_vec[:], t_vec[:], 1.0)

                    # recurrent matmuls, two contraction passes so the next
                    # step's pass A only waits on half 0 of the new h.
                    # Separate PSUM tiles: a start=True clears has_written
                    # bits bank-wide, so passes must not share a bank.
                    psgA_t = psgp.tile(
                        [128, KT_H, 4 * S], f32, tag="psgA", name="psgA_t"
                    )
                    psgB_t = psgp.tile(
                        [128, KT_H, 4 * S], f32, tag="psgB", name="psgB_t"
                    )
                    for m in range(MT):
                        g, kt = divmod(m, KT_H)
                        for k in range(3):
                            nc.tensor.matmul(
                                psgA_t[:, kt, ts(g, S)],
                                wt[:, KT_X + k, ts(m, 128)],
                                h_bf[:, k, :],
                                start=(k == 0),
                                stop=(k == 2),
                            )
                    for m in range(MT):
                        g, kt = divmod(m, KT_H)
                        for k in range(3, KT_H):
                            nc.tensor.matmul(
                                psgB_t[:, kt, ts(g, S)],
                                wt[:, KT_X + k, ts(m, 128)],
                                h_bf[:, k, :],
                                start=(k == 3),
                                stop=(k == KT_H - 1),
                            )

                    # fill the PE bubble: next stripe's xp m-groups
                    if xp_nxt is not None:
                        for m in range(24 * tl // ST, 24 * (tl + 1) // ST):
                            xp_mgroup(m, xp_nxt, xts_nxt)

                    # fill the PE bubble: transpose-flush of the previous stripe
                    if flush_src is not None and 2 <= tl <= 14:
                        u = tl - 2
                        if u < 12:
                            th, kt = u // 6, u % 6
                            if kt == 0:
                                osb[th] = osbp.tile(
                                    [128, H], f32, tag="osb", name="osb_t"
                                )
                            flush_unit(flush_src, osb[th], th, kt)
                            if kt == 5:
                                flush_dma(osb[th], flush_row + th * 8)

                    if NO_CHAIN:
                        continue
                    # elementwise chain, split into two k-halves so h_bf's
                    # first half is ready as early as possible
                    gt = workp.tile([128, KT_H, 4 * S], f32, tag="gt", name="gt_t")
                    ac = workp.tile([128, KT_H, 4 * S], f32, tag="ac", name="ac_t")
                    t1 = workp.tile([128, KT_H, S], f32, tag="t1", name="t1_t")
                    t2 = workp.tile([128, KT_H, S], f32, tag="t2", name="t2_t")
                    tct = workp.tile([128, KT_H, S], f32, tag="tct", name="tct_t")
                    ht = workp.tile([128, KT_H, S], f32, tag="ht", name="ht_t")
                    for hx in range(2):
                        kr = slice(3 * hx, 3 * hx + 3)
                        # gates = psumA + psumB + xp (bias folded into xp);
                        # each add reads at most one PSUM operand
                        nc.vector.tensor_add(
                            gt[:, kr, :], psgA_t[:, kr, :], xp_cur[:, kr, tl, :]
                        )
                        nc.vector.tensor_add(
                            gt[:, kr, :], psgB_t[:, kr, :], gt[:, kr, :]
                        )
                        nc.scalar.activation(
                            ac[:, kr, ts(0, S)], gt[:, kr, ts(0, S)], AF.Sigmoid
                        )
                        nc.scalar.activation(
                            ac[:, kr, ts(1, S)], gt[:, kr, ts(1, S)], AF.Tanh
                        )
                        nc.scalar.activation(
                            ac[:, kr, ds(2 * S, 2 * S)],
                            gt[:, kr, ds(2 * S, 2 * S)],
                            AF.Sigmoid,
                        )
                        # c = mask ? (c * f + i * j) : c
                        nc.vector.tensor_mul(
                            t1[:, kr, :], ac[:, kr, ts(0, S)], ac[:, kr, ts(1, S)]
                        )
                        nc.vector.tensor_mul(
                            t2[:, kr, :], c_sb[:, kr, :], ac[:, kr, ts(2, S)]
                        )
                        nc.vector.tensor_add(t2[:, kr, :], t2[:, kr, :], t1[:, kr, :])
                        nc.vector.copy_predicated(
                            c_sb[:, kr, :], mask[:, kr, :], t2[:, kr, :]
                        )
                        # h = mask ? tanh(c) * o : h (h_bf updated directly)
                        nc.scalar.activation(tct[:, kr, :], c_sb[:, kr, :], AF.Tanh)
                        nc.vector.tensor_mul(
                            ht[:, kr, :], tct[:, kr, :], ac[:, kr, ts(3, S)]
                        )
                        nc.vector.copy_predicated(
                            h_bf[:, kr, :], mask[:, kr, :], ht[:, kr, :]
                        )
                        nc.vector.copy_predicated(
                            out_tile[:, kr, tl, :], mask[:, kr, :], ht[:, kr, :]
                        )

            # Prologue: stripes 0,1 staged; xp for stripe 0
            load_xts(xtsA, 0)
            load_xts(xtsB, ST * S)
            for m in range(MT):
                xp_mgroup(m, xpA, xtsA)

            with tc.For_i(
                0,
                n_stripes,
                2,
                hint_engines=(ET.PE, ET.DVE, ET.Activation, ET.SP, ET.Pool),
            ) as iv:
                # iv = even stripe index a; stripes (a, a+1) this iteration.
                # xtsA holds x(a) (consumed last iter) -> refill with x(a+2)
                # now; xtsB (holding x(a+1)) is still needed by stripe a's
                # xp-build, so its refill with x(a+3) is emitted after.
                load_xts(xtsA, iv * (ST * S) + 2 * ST * S)
                # stripe a: consume xpA; build xp(a+1) in xpB from xtsB;
                # flush previous iteration's outB (stripe a-1) to rows
                # (a-1+1)*ST = iv*ST (iteration 0 writes the pad stripe).
                stripe_body(xpA, xpB, xtsB, outA, outB, iv * ST)
                load_xts(xtsB, iv * (ST * S) + 3 * ST * S)
                # stripe a+1: consume xpB; build xp(a+2) in xpA from the
                # refilled xtsA; flush outA (stripe a) to rows (a+1)*ST.
                stripe_body(xpB, xpA, xtsA, outB, outA, iv * ST + ST)

            # Epilogue: flush the last stripe (n_stripes-1, in outB) + state
            for th in range(2):
                osb_t = osbp.tile([128, H], f32, tag="osb", name="osb_t")
                for kt in range(KT_H):
                    flush_unit(outB, osb_t, th, kt)
                flush_dma(osb_t, n_stripes * ST + th * 8)


    nc.compile()
    return nc


def _get_program(t_steps):
    if t_steps not in _PROGRAM_CACHE:
        _PROGRAM_CACHE[t_steps] = _build_program(t_steps)
    return _PROGRAM_CACHE[t_steps]


def _reverse_sequence_np(x, seq_len):
    # mirrors tf.reverse_sequence along axis 1
    t = np.arange(x.shape[1])
    idx = np.where(
        t[None, :] < seq_len[:, None], seq_len[:, None] - 1 - t[None, :], t[None, :]
    )
    return np.take_along_axis(x, idx[:, :, None], axis=1)


def _core_inputs(x_blk, w_np, b_np, sl_blk, t_steps):
    """Build one core's input map from a [16, T, D] fp32 block."""
    bf = ml_dtypes.bfloat16
    # xts[p, k, t*16+s] = x[s, t, 128k+p]; pad 2 stripes of zeros in t
    xts = np.zeros((128, KT_X, (t_steps + 2 * ST) * S), dtype=bf)
    xts[:, :, : t_steps * S] = (
        x_blk.transpose(2, 1, 0).reshape(KT_X, 128, t_steps * S).transpose(1, 0, 2)
    ).astype(bf)
    # wt[p, k, g] = W[128k+p, g]
    import concourse.mybir as _mybir

    wnp_dt = _mybir.dt.np(getattr(_mybir.dt, W_DTYPE))
    wt = np.ascontiguousarray(
        w_np.reshape(KT_X + KT_H, 128, 4 * H).transpose(1, 0, 2)
    ).astype(wnp_dt)
    b_eff = b_np.astype(np.float64).copy()
    b_eff[2 * H : 3 * H] += FORGET_BIAS
    bcol = np.ascontiguousarray(b_eff.reshape(MT, 128).T).astype(np.float32)
    slen = np.broadcast_to(
        sl_blk.astype(np.float32)[None, None, :], (128, KT_H, S)
    ).copy()
    return {"wt": wt, "bcol": bcol, "slen": slen, "xts": xts}


_RUNNER_CACHE = {}


def _get_runner(t_steps):
    """Build (once) a jitted 8-core shard_map dispatcher for the program.

    Modeled on bass2jax.run_bass_via_pjrt, but the jitted callable and the
    donated-zero output operands are cached so repeat kernel() calls pay
    only input transfer + device execution, not XLA/NEFF recompilation.
    The kernel writes every output element, so donation is not needed.
    """
    if t_steps in _RUNNER_CACHE:
        return _RUNNER_CACHE[t_steps]

    import jax
    from jax.sharding import Mesh, PartitionSpec, NamedSharding
    from jax.experimental.shard_map import shard_map
    import concourse.mybir as mybir
    from concourse import bass2jax
    from concourse.bass2jax import _bass_exec_p, install_neuronx_cc_hook

    nc = _get_program(t_steps)
    install_neuronx_cc_hook()
    n_cores = 8

    partition_name = nc.partition_id_tensor.name if nc.partition_id_tensor else None
    in_names, out_names, out_avals, zero_outs = [], [], [], []
    for alloc in nc.m.functions[0].allocations:
        if not isinstance(alloc, mybir.MemoryLocationSet):
            continue
        name = alloc.memorylocations[0].name
        if alloc.kind == "ExternalInput":
            if name != partition_name:
                in_names.append(name)
        elif alloc.kind == "ExternalOutput":
            out_names.append(name)
            shape = tuple(alloc.tensor_shape)
            dtype = mybir.dt.np(alloc.dtype)
            out_avals.append(jax.core.ShapedArray(shape, dtype))
            zero_outs.append(np.zeros(shape, dtype))
    n_params = len(in_names)
    all_in_names = in_names + out_names

    def _body(*args):
        operands = list(args)
        if partition_name is not None:
            operands.append(bass2jax.partition_id_tensor())
            names = all_in_names + [partition_name]
        else:
            names = all_in_names
        outs = _bass_exec_p.bind(
            *operands,
            out_avals=tuple(out_avals),
            in_names=tuple(names),
            out_names=tuple(out_names),
            lowering_input_output_aliases=(),
            sim_require_finite=True,
            sim_require_nnan=True,
            nc=nc,
        )
        return tuple(outs)

    devices = jax.devices()[:n_cores]
    mesh = Mesh(np.asarray(devices), ("core",))
    sharded = jax.jit(
        shard_map(
            _body,
            mesh=mesh,
            in_specs=(PartitionSpec("core"),) * (n_params + len(out_names)),
            out_specs=(PartitionSpec("core"),) * len(out_names),
            check_rep=False,
        )
    )
    sh = NamedSharding(mesh, PartitionSpec("core"))
    zero_args = [
        jax.device_put(
            np.zeros((n_cores * z.shape[0], *z.shape[1:]), z.dtype), sh
        )
        for z in zero_outs
    ]

    def run(in_maps, raw=False):
        concat_in = [
            np.concatenate([in_maps[c][name] for c in range(n_cores)], axis=0)
            for name in in_names
        ]
        dev_in = [jax.device_put(a, sh) for a in concat_in]
        if raw:
            # benchmarking hook: async-dispatch against resident inputs
            return lambda: sharded(*dev_in, *zero_args)
        out_arrs = sharded(*dev_in, *zero_args)
        return [
            {
                name: np.asarray(out_arrs[i]).reshape(
                    n_cores, *out_avals[i].shape
                )[c]
                for i, name in enumerate(out_names)
            }
            for c in range(n_cores)
        ]

    _RUNNER_CACHE[t_steps] = run
    return run


def kernel(seq, seq_len, W_fw, b_fw, W_bw, b_bw):
    seq = np.asarray(seq, dtype=np.float32)
    seq_len = np.asarray(seq_len, dtype=np.int32)
    W_fw = np.asarray(W_fw, dtype=np.float32)
    b_fw = np.asarray(b_fw, dtype=np.float32)
    W_bw = np.asarray(W_bw, dtype=np.float32)
    b_bw = np.asarray(b_bw, dtype=np.float32)

    b_sz, t_steps, _ = seq.shape
    assert b_sz == B and seq.shape[2] == D

    run = _get_runner(t_steps)

    seq_rev = _reverse_sequence_np(seq, seq_len)

    in_maps = []
    for c in range(8):
        blk = slice(16 * (c % 4), 16 * (c % 4) + S)
        if c < 4:
            in_maps.append(_core_inputs(seq[blk], W_fw, b_fw, seq_len[blk], t_steps))
        else:
            in_maps.append(
                _core_inputs(seq_rev[blk], W_bw, b_bw, seq_len[blk], t_steps)
            )

    results = run(in_maps)

    outputs = np.zeros((B, t_steps, 2 * H), dtype=np.float32)
    state_h = np.zeros((B, 2 * H), dtype=np.float32)
    idx = np.arange(S)
    for c in range(8):
        blk = slice(16 * (c % 4), 16 * (c % 4) + S)
        o = results[c]["outd"][:, ST:, :]
        # final state == last valid output of the raw recurrence
        st = o[idx, seq_len[blk] - 1, :]
        if c < 4:
            outputs[blk, :, :H] = o
            state_h[blk, :H] = st
        else:
            outputs[blk, :, H:] = _reverse_sequence_np(o, seq_len[blk])
            state_h[blk, H:] = st
    return outputs, state_h
